# revision 3
# baseline (speedup 1.0000x reference)
"""Causal self-attention on 8 Trainium2 NeuronCores (Bass/Tile), bf16 PE path.

Problem: B=2, S=2048, D=1024, H=16 heads (hd=64), fp32 in/out.

Sharding (SPMD, same NEFF on 8 cores, different data):
  core c -> batch b = c//4, head-group g = c%4 (4 heads = 256 cols of wq/wk/wv,
  256 rows of wo). Each core computes its 4 heads' attention plus the partial
  output projection. Host sums the 4 partials per batch and adds bo.

All matmuls run in bf16 (fp32 PSUM accumulation): full-rate PE vs 4x-slower
fp32/f32r. Per-core dataflow (everything stays transposed; no on-device
transposes):
  qT/kT = matmul(lhsT=w[d,j], rhs=xT[d,s]) -> [j, s]   two-sweep k-split so
      the projections start while x still streams from HBM; sweep A evacuated
      by ScalarE (with bias), sweep B combined with a DVE add.
  v     = matmul(lhsT=xT[d,s], rhs=wv[d,j]) -> [s, j]; bias comes in via the
      sweep-A DVE evac add against a partition-broadcast bv row.
  scoresT[s_k, s_q] = matmul(lhsT=kT[j, s_k], rhs=qT[j, s_q])  (K=64, two
      heads row-packed into the PE via tile_position -> concurrent on HW)
  P = exp(scores/8) (ScalarE, PSUM->SBUF bf16); causal triangle of diagonal
      blocks zeroed post-exp by a GpSimd affine_select (off the PE/ACT path).
  yT[j, s_q] (+ column sums via a ones column in the stationary)
      = matmul(lhsT=[v|ones], rhs=P)  (K=128)
  normalize: DVE reciprocal of the sums row, PE row-64-ones broadcast matmul,
      DVE mul into yT (odd head hops partitions via SBUF-SBUF DMA).
  outT[e, s] = matmul(lhsT=wo[j, e], rhs=yT[j, s]) -> bf16 partials to HBM;
      host upcasts, sums the 4 partials per batch, adds bo.

Scheduling: phase-2 is software-pipelined (sc(kb+1) before av(kb)); the
sweep-B projection groups for later q-chunks are interleaved as PE fillers
into qc0's attention (they only gate qc1+), and each chunk's out-projection
is interleaved into the NEXT chunk's kb loop so only qc3's remains as tail.
Diagonal blocks are column-narrowed (512-128d query cols).
"""

import numpy as np

import concourse.bass as bass
import concourse.tile as tile
from concourse import bacc, mybir
from concourse.bass_utils import run_bass_kernel_spmd

P = 128
B, S, D, H, HD = 2, 2048, 1024, 16, 64
JH = 256          # head-dim columns per core (4 heads x 64)
KT = D // P       # 8 contraction tiles for the projections
QC = 512          # query-chunk (matmul moving free dim)
NQC = S // QC     # 4
NKB = S // P      # 16 key blocks
F32 = mybir.dt.float32
BF16 = mybir.dt.bfloat16
AF = mybir.ActivationFunctionType

_NC = None


def build(repeats: int = 1, num_devices: int = 8):
    nc = bacc.Bacc(
        "TRN2", target_bir_lowering=False, debug=False, num_devices=num_devices
    )

    # x and the big weights arrive pre-cast to bf16 from the host (the PE
    # consumes bf16 anyway — identical numerics, half the HBM traffic)
    xT_d = nc.dram_tensor("xT", [D, S], BF16, kind="ExternalInput").ap()
    wq_d = nc.dram_tensor("wq", [D, JH], BF16, kind="ExternalInput").ap()
    wk_d = nc.dram_tensor("wk", [D, JH], BF16, kind="ExternalInput").ap()
    wv_d = nc.dram_tensor("wv", [D, JH], BF16, kind="ExternalInput").ap()
    wo_d = nc.dram_tensor("wo", [JH, D], BF16, kind="ExternalInput").ap()
    bq_d = nc.dram_tensor("bq", [JH], F32, kind="ExternalInput").ap()
    bk_d = nc.dram_tensor("bk", [JH], F32, kind="ExternalInput").ap()
    bv_d = nc.dram_tensor("bv", [JH], F32, kind="ExternalInput").ap()
    outT_d = nc.dram_tensor("outT", [D, S], BF16, kind="ExternalOutput").ap()

    xT_re = xT_d.rearrange("(o p) s -> p o s", p=P)      # [128, 8, 2048]
    wq_re = wq_d.rearrange("(o p) j -> p o j", p=P)      # [128, 8, 256]
    wk_re = wk_d.rearrange("(o p) j -> p o j", p=P)
    wv_re = wv_d.rearrange("(o p) j -> p o j", p=P)
    wo_re = wo_d.rearrange("(o p) e -> p o e", p=P)      # [128, 2, 1024]
    bq_re = bq_d.rearrange("(t p) -> p t", p=P)          # [128, 2]
    bk_re = bk_d.rearrange("(t p) -> p t", p=P)
    outT_re = outT_d.rearrange("(o p) s -> p o s", p=P)  # [128, 8, 2048]

    with tile.TileContext(nc) as tc:
        with (
            tc.tile_pool(name="persist", bufs=1) as persist,
            tc.tile_pool(name="ps_a", bufs=2, space="PSUM") as ps_a,
            tc.tile_pool(name="ps_o", bufs=2, space="PSUM") as ps_o,
            tc.tile_pool(name="ps_yt", bufs=2, space="PSUM") as ps_yt,
        ):
            # ---------------- persistent SBUF ----------------
            qT = persist.tile([P, 2, S], BF16, tag="qT")
            kT = persist.tile([P, 2, S], BF16, tag="kT")
            # per (k-block, head): [v(0:64) | ones(64)]
            vsb = persist.tile([P, NKB, 4, 65], BF16, tag="vsb")
            yT = persist.tile([P, 2, S], BF16, tag="yT")
            wo_r = persist.tile([P, 2, D], BF16, tag="wo_r")
            bq_sb = persist.tile([P, 2], F32, tag="bq")
            bk_sb = persist.tile([P, 2], F32, tag="bk")
            bv_bc = persist.tile([P, 4, 64], BF16, tag="bvbc")
            # row-64-ones stationary + dedicated reciprocal-row tiles for the
            # PE-based sums broadcast (rows other than 64 stay zero forever;
            # GpSimd partition_broadcast can't source row 64 — HW reads p0)
            e64_r = persist.tile([P, P], BF16, tag="e64r")
            rtr0 = persist.tile([P, QC], BF16, tag="rtr0")
            rtr1 = persist.tile([P, QC], BF16, tag="rtr1")

            nc.sync.dma_start(bq_sb[:], bq_re)
            nc.sync.dma_start(bk_sb[:], bk_re)

            # constants init: everything on GpSimd so the DVE/ACT queues stay
            # free for the weight/x casts on the critical path
            with tc.tile_pool(name="initp", bufs=1) as initp:
                vproto = initp.tile([P, 2, 65], F32, tag="vproto")
                bvrow = initp.tile([P, JH], BF16, tag="bvrow")
                bvf = initp.tile([P, JH], F32, tag="bvf")
                e0_f = initp.tile([P, P], F32, tag="e0f")

                # zero recip tiles; e64: row 64 = ones (sums-broadcast mm)
                nc.gpsimd.memset(e0_f[:], 0.0)
                nc.gpsimd.tensor_copy(
                    rtr0[:], e0_f[:, 0:1].to_broadcast((P, QC))
                )
                nc.gpsimd.tensor_copy(
                    rtr1[:], e0_f[:, 0:1].to_broadcast((P, QC))
                )
                nc.gpsimd.memset(e0_f[64:65, :], 1.0)
                nc.gpsimd.tensor_copy(e64_r[:], e0_f[:])

                nc.sync.dma_start(bvf[0:1, :], bv_d[None, :])
                nc.gpsimd.tensor_copy(bvrow[0:1, :], bvf[0:1, :])
                nc.gpsimd.partition_broadcast(bvrow[:], bvrow[0:1, :])
                nc.gpsimd.tensor_copy(
                    bv_bc[:], bvrow[:].rearrange("p (h j) -> p h j", h=4)
                )

                # v prototype row: ones at col 64 ([v|ones] for every head)
                nc.gpsimd.memset(vproto[:], 0.0)
                nc.gpsimd.memset(vproto[:, 0, 64:65], 1.0)
                nc.gpsimd.tensor_copy(
                    vsb[:],
                    vproto[:, None, 0:1, :].to_broadcast((P, NKB, 4, 65)),
                )

            for _rep in range(repeats):
                with (
                    tc.tile_pool(name="ph1", bufs=1) as ph1,
                    tc.tile_pool(name="xstage", bufs=2) as xstage,
                    tc.tile_pool(name="wstage", bufs=2) as wstage_pool,
                    tc.tile_pool(name="pt_pool", bufs=4) as pt_pool,
                    tc.tile_pool(name="recip", bufs=2) as recip_pool,
                    tc.tile_pool(name="ostage", bufs=2) as ostage,
                ):
                    # ------------- phase 1: load, cast, project -------------
                    wq_r = ph1.tile([P, KT, JH], BF16, tag="wq_r")
                    wk_r = ph1.tile([P, KT, JH], BF16, tag="wk_r")
                    wv_r = ph1.tile([P, KT, JH], BF16, tag="wv_r")
                    xT_r = ph1.tile([P, KT, S], BF16, tag="xT_r")

                    # DMA order: wk, x0..3, wq, wv, x4..7, wo — sweep A's
                    # k-groups only need wk + x0..3, so the PE starts early.
                    wkst = wstage_pool.tile([P, KT, JH], F32, tag="wst")
                    nc.sync.dma_start(wkst[:], wk_re)
                    nc.vector.tensor_copy(wk_r[:], wkst[:])
                    for kt in range(4):
                        xs = xstage.tile([P, S], F32, tag="xs")
                        nc.sync.dma_start(xs[:], xT_re[:, kt, :])
                        # early x casts on ScalarE (idle before evacs start)
                        nc.scalar.copy(xT_r[:, kt, :], xs[:])
                    for w_re, w_r in ((wq_re, wq_r), (wv_re, wv_r)):
                        st = wstage_pool.tile([P, KT, JH], F32, tag="wst")
                        nc.sync.dma_start(st[:], w_re)
                        nc.vector.tensor_copy(w_r[:], st[:])
                    for kt in range(4, KT):
                        xs = xstage.tile([P, S], F32, tag="xs")
                        nc.sync.dma_start(xs[:], xT_re[:, kt, :])
                        # late x casts on DVE (ScalarE is busy with evacs)
                        nc.vector.tensor_copy(xT_r[:, kt, :], xs[:])
                    wost = wstage_pool.tile([P, KT, JH], F32, tag="wst")
                    nc.sync.dma_start(
                        wost[:].rearrange("p a b -> p (a b)").rearrange(
                            "p (a b) -> p a b", a=2
                        ),
                        wo_re,
                    )
                    nc.vector.tensor_copy(
                        wo_r[:],
                        wost[:].rearrange("p a b -> p (a b)").rearrange(
                            "p (a b) -> p a b", a=2
                        ),
                    )

                    HKT = KT // 2

                    def kq_groups():
                        for sc in range(NQC):
                            for w_r, bias_sb, dst in (
                                (wk_r, bk_sb, kT),
                                (wq_r, bq_sb, qT),
                            ):
                                for jt in range(2):
                                    yield jt, w_r, bias_sb, dst, sc

                    # sweep A: x k-tiles 0..3, kt-OUTER within a segment of
                    # two q-chunks so the first matmul only needs wk + x0
                    # (instead of x0..3) — the PE starts ~10us earlier while
                    # x still streams. Evacuated with bias by ScalarE.
                    for w_r, bias_sb, dst in ((wk_r, bk_sb, kT), (wq_r, bq_sb, qT)):
                        for scp in range(2):
                            for jt in range(2):
                                seg = ps_a.tile([P, 2, QC], F32, tag="a")
                                for kt in range(HKT):
                                    for si in range(2):
                                        sc = 2 * scp + si
                                        nc.tensor.matmul(
                                            seg[:, si, :],
                                            w_r[:, kt, jt * P : (jt + 1) * P],
                                            xT_r[:, kt, sc * QC : (sc + 1) * QC],
                                            start=(kt == 0),
                                            stop=(kt == HKT - 1),
                                        )
                                for si in range(2):
                                    sc = 2 * scp + si
                                    nc.scalar.activation(
                                        dst[:, jt, sc * QC : (sc + 1) * QC],
                                        seg[:, si, :],
                                        AF.Identity,
                                        bias=bias_sb[:, jt : jt + 1],
                                    )
                    for st_i in range(NKB):
                        acc = ps_o.tile([P, QC], F32, tag="o")
                        va = acc[:, 0:JH]
                        for kt in range(HKT):
                            nc.tensor.matmul(
                                va,
                                xT_r[:, kt, st_i * P : (st_i + 1) * P],
                                wv_r[:, kt, :],
                                start=(kt == 0),
                                stop=(kt == HKT - 1),
                            )
                        vsl = vsb[:, st_i, :, 0:64]
                        nc.vector.tensor_add(
                            vsl, va.rearrange("p (h j) -> p h j", h=4), bv_bc[:]
                        )

                    # sweep B emitters (combined into dst with a DVE add)
                    def kq_b(jt, w_r, bias_sb, dst, sc):
                        acc = ps_o.tile([P, QC], F32, tag="o")
                        for kt in range(HKT, KT):
                            nc.tensor.matmul(
                                acc,
                                w_r[:, kt, jt * P : (jt + 1) * P],
                                xT_r[:, kt, sc * QC : (sc + 1) * QC],
                                start=(kt == HKT),
                                stop=(kt == KT - 1),
                            )
                        dsl = dst[:, jt, sc * QC : (sc + 1) * QC]
                        nc.vector.tensor_add(dsl, acc, dsl)

                    def v_b(st_i):
                        acc = ps_o.tile([P, QC], F32, tag="o")
                        va = acc[:, 0:JH]
                        for kt in range(HKT, KT):
                            nc.tensor.matmul(
                                va,
                                xT_r[:, kt, st_i * P : (st_i + 1) * P],
                                wv_r[:, kt, :],
                                start=(kt == HKT),
                                stop=(kt == KT - 1),
                            )
                        vsl = vsb[:, st_i, :, 0:64]
                        nc.vector.tensor_add(
                            vsl, va.rearrange("p (h j) -> p h j", h=4), vsl
                        )

                    kq_list = list(kq_groups())
                    # qc0 needs kq sc0 + v st0..3: emit those now; the rest
                    # become PE fillers inside qc0's attention loop.
                    for args in kq_list[:4]:
                        kq_b(*args)
                    for st_i in range(4):
                        v_b(st_i)
                    fillers = [(kq_b, args) for args in kq_list[4:]]
                    fillers += [(v_b, (st_i,)) for st_i in range(4, NKB)]
                    fillers.reverse()  # pop() from the front

                    # ------------- phase 2: attention + out-proj -------------
                    pending_oproj = []

                    def emit_oproj_unit(qc, et):
                        acc = ps_o.tile([P, QC], F32, tag="o")
                        for pair in range(2):
                            nc.tensor.matmul(
                                acc,
                                wo_r[:, pair, et * P : (et + 1) * P],
                                yT[:, pair, qc * QC : (qc + 1) * QC],
                                start=(pair == 0),
                                stop=(pair == 1),
                            )
                        ot = ostage.tile([P, QC], BF16, tag="ot")
                        nc.vector.tensor_copy(ot[:], acc)
                        nc.sync.dma_start(
                            outT_re[:, et, qc * QC : (qc + 1) * QC], ot[:]
                        )

                    def drain(n):
                        for _ in range(n):
                            if fillers:
                                fn, args = fillers.pop()
                                fn(*args)
                            elif pending_oproj:
                                emit_oproj_unit(*pending_oproj.pop(0))
                            else:
                                break

                    for qc in range(NQC):
                        for pair in range(2):
                            nkb = 4 * (qc + 1)
                            y0 = ps_yt.tile([P, QC], F32, tag="yt")
                            y1 = ps_yt.tile([P, QC], F32, tag="yt")

                            def emit_scores(kb):
                                d = kb - 4 * qc  # >= 0 on diagonal blocks
                                n_d = QC - 128 * d if d > 0 else QC
                                q_off = qc * QC + (QC - n_d)
                                sc_ps = ps_a.tile([P, 2, QC], F32, tag="a")
                                for he in range(2):
                                    nc.tensor.matmul(
                                        sc_ps[:, he, 0:n_d],
                                        kT[64 * he : 64 * he + 64, pair,
                                           kb * P : (kb + 1) * P],
                                        qT[64 * he : 64 * he + 64, pair,
                                           q_off : q_off + n_d],
                                        start=True,
                                        stop=True,
                                        tile_position=(64 * he, 0),
                                    )
                                pt = pt_pool.tile([P, 2, QC], BF16, tag="pt")
                                nc.scalar.activation(
                                    pt[:, :, 0:n_d],
                                    sc_ps[:, :, 0:n_d],
                                    AF.Exp,
                                    scale=0.125,
                                )
                                if d >= 0:
                                    # zero the causal triangle post-exp
                                    # (GpSimd — keeps PE/ACT unburdened)
                                    for he in range(2):
                                        nc.gpsimd.affine_select(
                                            out=pt[:, he, 0:n_d],
                                            in_=pt[:, he, 0:n_d],
                                            compare_op=mybir.AluOpType.is_ge,
                                            fill=0.0, base=0,
                                            pattern=[[1, n_d]],
                                            channel_multiplier=-1,
                                        )
                                return pt, n_d

                            def emit_av(kb, pt, n_d):
                                h0 = 2 * pair
                                for he, yps in ((0, y0), (1, y1)):
                                    nc.tensor.matmul(
                                        yps[0:65, QC - n_d :],
                                        vsb[:, kb, h0 + he, :],
                                        pt[:, he, 0:n_d],
                                        start=(kb == 0),
                                        stop=(kb == nkb - 1),
                                    )

                            # software pipeline: sc(kb+1) issues before av(kb)
                            prev = None
                            for kb in range(nkb):
                                cur = (kb, *emit_scores(kb))
                                if prev is not None:
                                    emit_av(*prev)
                                prev = cur
                                drain(3 if qc == 0 else 1)
                            emit_av(*prev)

                            # normalize: y /= column-sums (sums at row 64).
                            # Evacuate y to SBUF (frees the PSUM accumulator
                            # slots), reciprocal of the sums row, broadcast
                            # across partitions with a PE matmul (lhsT =
                            # row-64-ones), DVE mul (SBUF x PSUM — legal).
                            ysg0 = recip_pool.tile([P, QC], F32, tag="ysg0")
                            ysg1 = recip_pool.tile([P, QC], F32, tag="ysg1")
                            nc.vector.tensor_copy(ysg0[0:65, :], y0[0:65, :])
                            nc.vector.tensor_copy(ysg1[0:65, :], y1[0:65, :])
                            with nc.allow_low_precision(
                                reason="bf16 reciprocal row for sums broadcast"
                            ):
                                nc.vector.reciprocal(
                                    rtr0[64:65, :], ysg0[64:65, :]
                                )
                                nc.vector.reciprocal(
                                    rtr1[64:65, :], ysg1[64:65, :]
                                )
                            rbp0 = ps_o.tile([P, QC], F32, tag="o")
                            nc.tensor.matmul(
                                rbp0[:], e64_r[:], rtr0[:], start=True, stop=True
                            )
                            nc.vector.tensor_mul(
                                yT[0:64, pair, qc * QC : (qc + 1) * QC],
                                ysg0[0:64, :],
                                rbp0[0:64, :],
                            )
                            ymid = recip_pool.tile([P, QC], F32, tag="ymid")
                            nc.sync.dma_start(ymid[64:128, :], ysg1[0:64, :])
                            rbp1 = ps_o.tile([P, QC], F32, tag="o")
                            nc.tensor.matmul(
                                rbp1[:], e64_r[:], rtr1[:], start=True, stop=True
                            )
                            nc.vector.tensor_mul(
                                yT[64:128, pair, qc * QC : (qc + 1) * QC],
                                ymid[64:128, :],
                                rbp1[64:128, :],
                            )

                        pending_oproj.extend((qc, et) for et in range(KT))

                    # remaining out-projection (qc3 + anything not drained)
                    drain(len(fillers) + len(pending_oproj))

    nc.compile()
    return nc


def _get_nc():
    global _NC
    if _NC is None:
        _NC = build()
    return _NC


def kernel(x, wq, bq, wk, bk, wv, bv, wo, bo, **run_kwargs):
    x = np.asarray(x, dtype=np.float32)
    wq = np.asarray(wq, dtype=np.float32)
    bq = np.asarray(bq, dtype=np.float32)
    wk = np.asarray(wk, dtype=np.float32)
    bk = np.asarray(bk, dtype=np.float32)
    wv = np.asarray(wv, dtype=np.float32)
    bv = np.asarray(bv, dtype=np.float32)
    wo = np.asarray(wo, dtype=np.float32)
    bo = np.asarray(bo, dtype=np.float32)

    nc = _get_nc()
    in_maps = []
    for c in range(8):
        b, g = divmod(c, 4)
        jsl = slice(JH * g, JH * (g + 1))
        in_maps.append(
            {
                "xT": np.ascontiguousarray(x[b].T),
                "wq": np.ascontiguousarray(wq[:, jsl]),
                "wk": np.ascontiguousarray(wk[:, jsl]),
                "wv": np.ascontiguousarray(wv[:, jsl]),
                "wo": np.ascontiguousarray(wo[jsl, :]),
                "bq": np.ascontiguousarray(bq[jsl]),
                "bk": np.ascontiguousarray(bk[jsl]),
                "bv": np.ascontiguousarray(bv[jsl]),
            }
        )
    res = run_bass_kernel_spmd(nc, in_maps, core_ids=list(range(8)), **run_kwargs)
    outs = [np.asarray(r["outT"], dtype=np.float32) for r in res.results]
    y = np.empty((B, S, D), dtype=np.float32)
    for b in range(B):
        acc = outs[4 * b] + outs[4 * b + 1] + outs[4 * b + 2] + outs[4 * b + 3]
        y[b] = acc.T + bo[None, :]
    if run_kwargs:
        kernel.last_result = res
    return y
